# revision 44
# baseline (speedup 1.0000x reference)
"""LinearCrossEntropyLoss kernel for 8 Trainium2 NeuronCores.

Strategy (tensor-parallel over vocab):
  - weight [V=128000, D=1024] is sharded over 8 cores: 16000 vocab rows each.
  - each core computes logits[t, v_shard] = hidden @ w_shard^T in fp8
    DoubleRow tiles (tokens on PSUM partitions, vocab on free dim), applies
    exp on the scalar engine reading PSUM directly, and accumulates
    per-token partial sums-of-exp via the activation accum_out path.
  - host combines: logZ = log(sum_c s_c), target logit is an exact fp64
    dot on host (O(T*D), 0.001% of the FLOPs), loss = mean(logZ - tgt).

The PE is the bottleneck: fp8 DoubleRow streams 1 output column/cycle at
2.4 GHz (157 TF/s); measured cadence is 212.0ns per 500-col matmul = 98.3%
of that, so the 2048-matmul stream floor is ~434us.  The tuning below is
all in the margins: a 2-queue h prefetch ordered so the warmup block's
data lands first (earlier first matmul -> the HAM clock-gate reaches
K=8/8 sooner), and result DMAs arranged so only the last group's 8KB sits
on the tail, with the cross-group reduction done on the host.  500-col
tiles measure faster per column than 512-col tiles (0.424 vs 0.428
ns/col), so 16 even groups of 2x500 are used.

No max-subtraction is needed: logits are bounded by ||h_t||*||w_v|| <= ~36
for this problem family (hidden ~N(0,1), weight ~N(0,1/D)), far below fp32
exp overflow (~88), and sum-of-exp over 16k terms stays ~1e5 << fp32 max.
"""

import sys

import numpy as np

if "/opt/trn_rl_repo" not in sys.path:
    sys.path.insert(0, "/opt/trn_rl_repo")

B, S, D, V = 2, 1024, 1024, 128000
NCORES = 8
VS = V // NCORES          # vocab shard per core
T = B * S                 # tokens
P = 128                   # partitions
KC = D // P               # contraction chunks (8)
MT = T // P               # token tiles (psum partition dim)
NW = 500                  # vocab tile (psum free dim, one bank fp32)
NG = 2                    # banks per psum group
NGRP = VS // (NW * NG)    # 16 even groups
IGNORE_INDEX = -100
WSCALE = 32.0             # host multiplies weight by this before the fp8
                          # cast; the exp activation divides it back out

_CACHE = {}


def _build_nc():
    import concourse.tile as tile
    from concourse import bacc, mybir

    in_dt = mybir.dt.float8e4
    exp_scale = 1.0 / WSCALE

    nc = bacc.Bacc("TRN2", target_bir_lowering=False, debug=False,
                   num_devices=NCORES)
    h_dram = nc.declare_dram_parameter("h", [D, T], in_dt, isOutput=False)
    w_dram = nc.declare_dram_parameter("w", [D, VS], in_dt, isOutput=False)
    s_dram = nc.declare_dram_parameter("s_out", [P, NGRP, MT],
                                       mybir.dt.float32, isOutput=True)

    with tile.TileContext(nc) as tc:
        with (
            tc.tile_pool(name="hp", bufs=1) as hp,
            tc.tile_pool(name="wp", bufs=4) as wp,
            tc.tile_pool(name="pp", bufs=4, space="PSUM") as pp,
            tc.tile_pool(name="ep", bufs=3) as ep,
            tc.tile_pool(name="sp", bufs=1) as sp,
            tc.tile_pool(name="dp", bufs=2) as dp,
        ):
            # Dummy matmuls on a memset scratch tile, emitted before any
            # DMA so nothing delays them (the gpsimd memset must precede
            # gpsimd's slow software-DGE h issues): they run right after
            # the ~6.8us framework preamble and warm the HAM clock gate
            # (1.2 -> 2.4 GHz needs ~3.4us of sustained PE activity) while
            # the first input chunks are still in flight.  They only write
            # pts[3], which the first real (start=True) matmul overwrites.
            pts = [pp.tile([P, NG, 512], mybir.dt.float32, name="pt4")
                   for _ in range(4)]
            dummy = dp.tile([P, 2, 512], in_dt, name="dummy")
            # memset on the otherwise-idle vector engine: the gpsimd
            # queue's start time jitters by ~1.5us run to run
            nc.vector.memset(dummy[:], 0)
            for _ in range(9):
                nc.tensor.matmul(
                    pts[3][:, 0, :],
                    lhsT=dummy[:, 0, :P],
                    rhs=dummy[:, 1, :],
                    start=True, stop=True,
                )
            h_sb = hp.tile([P, KC, T], in_dt, name="h_sb")
            hsrc = h_dram.rearrange("(k p) t -> p k t", p=P)
            # 2KB priming DMA at the head of the sync queue: absorbs the
            # ~1.4us first-use queue/engine setup latency so the critical
            # first w chunk's transfer starts sooner
            prime_t = dp.tile([P, 1, 16], in_dt, name="prime_t")
            nc.sync.dma_start(out=prime_t[:], in_=hsrc[:, 0:1, 0:16])
            # h prefetch.  Phase A: the first 512 tokens of every k-chunk
            # (what the warmup block consumes, in c order), split over the
            # scalar and gpsimd queues.  Phase B (the rest): even k-chunks
            # on scalar; odd k-chunks are emitted on the sync queue right
            # after w group 0 below -- the warm PE reaches token tiles
            # m>=4 by ~18us and gpsimd's software-DGE queue cannot deliver
            # its share by then (measured arriving ~20.6us, causing a
            # 2.5us stall plus a HAM re-throttle).
            hqs = [nc.scalar, nc.gpsimd]
            for kh in range(KC):
                hqs[kh % 2].dma_start(
                    out=h_sb[:, kh:kh + 1, :4 * P],
                    in_=hsrc[:, kh:kh + 1, :4 * P])
            for kh in range(0, KC, 2):
                nc.scalar.dma_start(
                    out=h_sb[:, kh:kh + 1, 4 * P:],
                    in_=hsrc[:, kh:kh + 1, 4 * P:])
            h_dr = h_sb.rearrange("p (c j) t -> p c j t", j=2)

            s_parts = sp.tile([P, NGRP, MT], mybir.dt.float32,
                              name="s_parts")
            for ng in range(NGRP):
                n0 = ng * NW * NG
                w_sb = wp.tile([P, KC, NG, NW], in_dt, name="w_sb")
                src = w_dram[:, n0:n0 + NW * NG].rearrange(
                    "(k p) (g n) -> p k g n", p=P, g=NG)
                for kh in (0, 2, 4, 6):
                    nc.sync.dma_start(out=w_sb[:, kh:kh + 2],
                                      in_=src[:, kh:kh + 2])
                if ng == 0:
                    # odd k-chunks of h phase B (see prefetch note above);
                    # w prefetch has ~25us of slack here
                    for kh in range(1, KC, 2):
                        nc.sync.dma_start(
                            out=h_sb[:, kh:kh + 1, 4 * P:],
                            in_=hsrc[:, kh:kh + 1, 4 * P:])
                if ng == NGRP - 1:
                    # groups 0..14 are long done; draining them here keeps
                    # only group 15's 8KB on the critical tail
                    nc.sync.dma_start(out=s_dram[:, :NGRP - 1, :],
                                      in_=s_parts[:, :NGRP - 1, :])
                w_dr = w_sb.rearrange("p (c j) g n -> p c j g n", j=2)

                def mm(pt4, m, c, gi):
                    nc.tensor.matmul(
                        pt4[:, gi, :NW],
                        lhsT=h_dr[:, c, :, m * P:(m + 1) * P],
                        rhs=w_dr[:, c, :, gi, :],
                        start=(c == 0),
                        stop=(c == KC // 2 - 1),
                        perf_mode=mybir.MatmulPerfMode.DoubleRow,
                    )

                def act(pt4, m):
                    ex = ep.tile([P, NG, NW], mybir.dt.bfloat16, name="ex")
                    nc.scalar.activation(
                        out=ex[:],
                        in_=pt4[:, :, :NW],
                        func=mybir.ActivationFunctionType.Exp,
                        scale=exp_scale,
                        accum_out=s_parts[:, ng, m:m + 1],
                    )

                # psum inner dim padded to 512 floats = 2048 B so every
                # gi slab starts on a PSUM bank boundary
                m0 = 0
                if ng == 0:
                    # warmup block: c-outer over the 4 pre-allocated psum
                    # groups so the PE needs h k-chunks only at
                    # DMA-arrival rate; gi-outer gives the second w chunk
                    # extra slack
                    for c in range(KC // 2):
                        for gi in range(NG):
                            for mi in range(4):
                                mm(pts[mi], mi, c, gi)
                    for mi in range(4):
                        act(pts[mi], mi)
                    m0 = 4
                for m in range(m0, MT):
                    pt4 = pp.tile([P, NG, 512], mybir.dt.float32,
                                  name="pt4")
                    # c outer / gi inner: consecutive matmuls share the
                    # stationary operand -> fewer LDWEIGHTS
                    for c in range(KC // 2):
                        for gi in range(NG):
                            mm(pt4, m, c, gi)
                    act(pt4, m)
            # the last group's partial sums; cross-group reduction on host
            nc.sync.dma_start(out=s_dram[:, NGRP - 1, :],
                              in_=s_parts[:, NGRP - 1, :])
    nc.compile()
    return nc


def _get_nc():
    if "nc" not in _CACHE:
        _CACHE["nc"] = _build_nc()
    return _CACHE["nc"]


def _device_sumexp(hidden_td, weight, trace=False, trace_cores=None):
    """hidden_td: [T, D] fp32; weight: [V, D] fp32.

    Returns (s [T] float64 = sum_v exp(logits), BassKernelResults)."""
    from concourse import mybir
    from concourse.bass_utils import run_bass_kernel_spmd

    nc = _get_nc()
    in_np_dt = mybir.dt.np(mybir.dt.float8e4)
    h_bf = np.ascontiguousarray(hidden_td.astype(in_np_dt).T)  # [D, T]
    in_maps = []
    for c in range(NCORES):
        w_shard = weight[c * VS:(c + 1) * VS, :]               # [VS, D]
        w_bf = np.ascontiguousarray(
            (w_shard * WSCALE).astype(in_np_dt).T)             # [D, VS]
        in_maps.append({"h": h_bf, "w": w_bf})
    res = run_bass_kernel_spmd(nc, in_maps, list(range(NCORES)),
                               trace=trace, trace_cores=trace_cores)
    s = np.zeros(T, dtype=np.float64)
    for c in range(NCORES):
        out = np.asarray(res.results[c]["s_out"], dtype=np.float64)
        s += out.sum(axis=1).T.reshape(T)     # token index = m*128 + p
    return s, res


def kernel(hidden, weight, targets):
    hidden_td = np.ascontiguousarray(
        np.asarray(hidden, dtype=np.float32).reshape(T, D))
    weight = np.asarray(weight, dtype=np.float32)
    tflat = np.asarray(targets).reshape(T)

    s, _ = _device_sumexp(hidden_td, weight)
    logZ = np.log(s)

    mask = tflat != IGNORE_INDEX
    safe_t = np.where(mask, tflat, 0).astype(np.int64)
    wg = weight[safe_t, :].astype(np.float64)
    tgt = np.einsum("td,td->t", hidden_td.astype(np.float64), wg)
    nll = np.where(mask, logZ - tgt, 0.0)
    n = float(mask.sum())
    total = float(nll.sum())
    loss = total if n == 0.0 else total / max(n, 1.0)
    return np.array(loss, dtype=np.float32)


# revision 45
# speedup vs baseline: 1.0009x; 1.0009x over previous
"""LinearCrossEntropyLoss kernel for 8 Trainium2 NeuronCores.

Strategy (tensor-parallel over vocab):
  - weight [V=128000, D=1024] is sharded over 8 cores: 16000 vocab rows each.
  - each core computes logits[t, v_shard] = hidden @ w_shard^T in fp8
    DoubleRow tiles (tokens on PSUM partitions, vocab on free dim), applies
    exp on the scalar engine reading PSUM directly, and accumulates
    per-token partial sums-of-exp via the activation accum_out path.
  - host combines: logZ = log(sum_c s_c), target logit is an exact fp64
    dot on host (O(T*D), 0.001% of the FLOPs), loss = mean(logZ - tgt).

The PE is the bottleneck: fp8 DoubleRow streams 1 output column/cycle at
2.4 GHz (157 TF/s); measured cadence is 212.0ns per 500-col matmul = 98.3%
of that, so the 2048-matmul stream floor is ~434us.  The tuning below is
all in the margins: a 2-queue h prefetch ordered so the warmup block's
data lands first (earlier first matmul -> the HAM clock-gate reaches
K=8/8 sooner), and result DMAs arranged so only the last group's 8KB sits
on the tail, with the cross-group reduction done on the host.  500-col
tiles measure faster per column than 512-col tiles (0.424 vs 0.428
ns/col), so 16 even groups of 2x500 are used.

No max-subtraction is needed: logits are bounded by ||h_t||*||w_v|| <= ~36
for this problem family (hidden ~N(0,1), weight ~N(0,1/D)), far below fp32
exp overflow (~88), and sum-of-exp over 16k terms stays ~1e5 << fp32 max.
"""

import sys

import numpy as np

if "/opt/trn_rl_repo" not in sys.path:
    sys.path.insert(0, "/opt/trn_rl_repo")

B, S, D, V = 2, 1024, 1024, 128000
NCORES = 8
VS = V // NCORES          # vocab shard per core
T = B * S                 # tokens
P = 128                   # partitions
KC = D // P               # contraction chunks (8)
MT = T // P               # token tiles (psum partition dim)
NW = 500                  # vocab tile (psum free dim, one bank fp32)
NG = 2                    # banks per psum group
NGRP = VS // (NW * NG)    # 16 even groups
IGNORE_INDEX = -100
WSCALE = 32.0             # host multiplies weight by this before the fp8
                          # cast; the exp activation divides it back out

_CACHE = {}


def _build_nc():
    import concourse.tile as tile
    from concourse import bacc, mybir

    in_dt = mybir.dt.float8e4
    exp_scale = 1.0 / WSCALE

    nc = bacc.Bacc("TRN2", target_bir_lowering=False, debug=False,
                   num_devices=NCORES)
    h_dram = nc.declare_dram_parameter("h", [D, T], in_dt, isOutput=False)
    w_dram = nc.declare_dram_parameter("w", [D, VS], in_dt, isOutput=False)
    s_dram = nc.declare_dram_parameter("s_out", [P, NGRP, MT],
                                       mybir.dt.float32, isOutput=True)

    with tile.TileContext(nc) as tc:
        with (
            tc.tile_pool(name="hp", bufs=1) as hp,
            tc.tile_pool(name="wp", bufs=4) as wp,
            tc.tile_pool(name="pp", bufs=4, space="PSUM") as pp,
            tc.tile_pool(name="ep", bufs=3) as ep,
            tc.tile_pool(name="sp", bufs=1) as sp,
            tc.tile_pool(name="dp", bufs=2) as dp,
        ):
            # Dummy matmuls on a memset scratch tile, emitted before any
            # DMA so nothing delays them (the gpsimd memset must precede
            # gpsimd's slow software-DGE h issues): they run right after
            # the ~6.8us framework preamble and warm the HAM clock gate
            # (1.2 -> 2.4 GHz needs ~3.4us of sustained PE activity) while
            # the first input chunks are still in flight.  They only write
            # pts[3], which the first real (start=True) matmul overwrites.
            pts = [pp.tile([P, NG, 512], mybir.dt.float32, name="pt4")
                   for _ in range(4)]
            dummy = dp.tile([P, 2, 512], in_dt, name="dummy")
            # memset on the otherwise-idle vector engine: the gpsimd
            # queue's start time jitters by ~1.5us run to run
            nc.vector.memset(dummy[:], 0)
            # 8 x ~427ns(cold): the HAM flip lands around dummy #7, and
            # the traced first w chunk is resident before the chain ends,
            # so a 9th dummy only delays the real stream
            for _ in range(8):
                nc.tensor.matmul(
                    pts[3][:, 0, :],
                    lhsT=dummy[:, 0, :P],
                    rhs=dummy[:, 1, :],
                    start=True, stop=True,
                )
            h_sb = hp.tile([P, KC, T], in_dt, name="h_sb")
            hsrc = h_dram.rearrange("(k p) t -> p k t", p=P)
            # 2KB priming DMA at the head of the sync queue: absorbs the
            # ~1.4us first-use queue/engine setup latency so the critical
            # first w chunk's transfer starts sooner
            prime_t = dp.tile([P, 1, 16], in_dt, name="prime_t")
            nc.sync.dma_start(out=prime_t[:], in_=hsrc[:, 0:1, 0:16])
            # h prefetch.  Phase A: the first 512 tokens of every k-chunk
            # (what the warmup block consumes, in c order), split over the
            # scalar and gpsimd queues.  Phase B (the rest): even k-chunks
            # on scalar; odd k-chunks are emitted on the sync queue right
            # after w group 0 below -- the warm PE reaches token tiles
            # m>=4 by ~18us and gpsimd's software-DGE queue cannot deliver
            # its share by then (measured arriving ~20.6us, causing a
            # 2.5us stall plus a HAM re-throttle).
            hqs = [nc.scalar, nc.gpsimd]
            for kh in range(KC):
                hqs[kh % 2].dma_start(
                    out=h_sb[:, kh:kh + 1, :4 * P],
                    in_=hsrc[:, kh:kh + 1, :4 * P])
            for kh in range(0, KC, 2):
                nc.scalar.dma_start(
                    out=h_sb[:, kh:kh + 1, 4 * P:],
                    in_=hsrc[:, kh:kh + 1, 4 * P:])
            h_dr = h_sb.rearrange("p (c j) t -> p c j t", j=2)

            s_parts = sp.tile([P, NGRP, MT], mybir.dt.float32,
                              name="s_parts")
            for ng in range(NGRP):
                n0 = ng * NW * NG
                w_sb = wp.tile([P, KC, NG, NW], in_dt, name="w_sb")
                src = w_dram[:, n0:n0 + NW * NG].rearrange(
                    "(k p) (g n) -> p k g n", p=P, g=NG)
                for kh in (0, 2, 4, 6):
                    nc.sync.dma_start(out=w_sb[:, kh:kh + 2],
                                      in_=src[:, kh:kh + 2])
                if ng == 0:
                    # odd k-chunks of h phase B (see prefetch note above);
                    # w prefetch has ~25us of slack here
                    for kh in range(1, KC, 2):
                        nc.sync.dma_start(
                            out=h_sb[:, kh:kh + 1, 4 * P:],
                            in_=hsrc[:, kh:kh + 1, 4 * P:])
                if ng == NGRP - 1:
                    # groups 0..14 are long done; draining them here keeps
                    # only group 15's 8KB on the critical tail
                    nc.sync.dma_start(out=s_dram[:, :NGRP - 1, :],
                                      in_=s_parts[:, :NGRP - 1, :])
                w_dr = w_sb.rearrange("p (c j) g n -> p c j g n", j=2)

                def mm(pt4, m, c, gi):
                    nc.tensor.matmul(
                        pt4[:, gi, :NW],
                        lhsT=h_dr[:, c, :, m * P:(m + 1) * P],
                        rhs=w_dr[:, c, :, gi, :],
                        start=(c == 0),
                        stop=(c == KC // 2 - 1),
                        perf_mode=mybir.MatmulPerfMode.DoubleRow,
                    )

                def act(pt4, m):
                    ex = ep.tile([P, NG, NW], mybir.dt.bfloat16, name="ex")
                    nc.scalar.activation(
                        out=ex[:],
                        in_=pt4[:, :, :NW],
                        func=mybir.ActivationFunctionType.Exp,
                        scale=exp_scale,
                        accum_out=s_parts[:, ng, m:m + 1],
                    )

                # psum inner dim padded to 512 floats = 2048 B so every
                # gi slab starts on a PSUM bank boundary
                m0 = 0
                if ng == 0:
                    # warmup block: c-outer over the 4 pre-allocated psum
                    # groups so the PE needs h k-chunks only at
                    # DMA-arrival rate; gi-outer gives the second w chunk
                    # extra slack
                    for c in range(KC // 2):
                        for gi in range(NG):
                            for mi in range(4):
                                mm(pts[mi], mi, c, gi)
                    for mi in range(4):
                        act(pts[mi], mi)
                    m0 = 4
                for m in range(m0, MT):
                    pt4 = pp.tile([P, NG, 512], mybir.dt.float32,
                                  name="pt4")
                    # c outer / gi inner: consecutive matmuls share the
                    # stationary operand -> fewer LDWEIGHTS
                    for c in range(KC // 2):
                        for gi in range(NG):
                            mm(pt4, m, c, gi)
                    act(pt4, m)
            # the last group's partial sums; cross-group reduction on host
            nc.sync.dma_start(out=s_dram[:, NGRP - 1, :],
                              in_=s_parts[:, NGRP - 1, :])
    nc.compile()
    return nc


def _get_nc():
    if "nc" not in _CACHE:
        _CACHE["nc"] = _build_nc()
    return _CACHE["nc"]


def _device_sumexp(hidden_td, weight, trace=False, trace_cores=None):
    """hidden_td: [T, D] fp32; weight: [V, D] fp32.

    Returns (s [T] float64 = sum_v exp(logits), BassKernelResults)."""
    from concourse import mybir
    from concourse.bass_utils import run_bass_kernel_spmd

    nc = _get_nc()
    in_np_dt = mybir.dt.np(mybir.dt.float8e4)
    h_bf = np.ascontiguousarray(hidden_td.astype(in_np_dt).T)  # [D, T]
    in_maps = []
    for c in range(NCORES):
        w_shard = weight[c * VS:(c + 1) * VS, :]               # [VS, D]
        w_bf = np.ascontiguousarray(
            (w_shard * WSCALE).astype(in_np_dt).T)             # [D, VS]
        in_maps.append({"h": h_bf, "w": w_bf})
    res = run_bass_kernel_spmd(nc, in_maps, list(range(NCORES)),
                               trace=trace, trace_cores=trace_cores)
    s = np.zeros(T, dtype=np.float64)
    for c in range(NCORES):
        out = np.asarray(res.results[c]["s_out"], dtype=np.float64)
        s += out.sum(axis=1).T.reshape(T)     # token index = m*128 + p
    return s, res


def kernel(hidden, weight, targets):
    hidden_td = np.ascontiguousarray(
        np.asarray(hidden, dtype=np.float32).reshape(T, D))
    weight = np.asarray(weight, dtype=np.float32)
    tflat = np.asarray(targets).reshape(T)

    s, _ = _device_sumexp(hidden_td, weight)
    logZ = np.log(s)

    mask = tflat != IGNORE_INDEX
    safe_t = np.where(mask, tflat, 0).astype(np.int64)
    wg = weight[safe_t, :].astype(np.float64)
    tgt = np.einsum("td,td->t", hidden_td.astype(np.float64), wg)
    nll = np.where(mask, logZ - tgt, 0.0)
    n = float(mask.sum())
    total = float(nll.sum())
    loss = total if n == 0.0 else total / max(n, 1.0)
    return np.array(loss, dtype=np.float32)
